# revision 31
# baseline (speedup 1.0000x reference)
"""Trainium2 Bass kernel for the dense_cnn problem (1x1 conv + BN + SiLU ->
attention-weighted dynamic 3x3 conv with instance norm), data-parallel over
batch across 8 NeuronCores.

Self-contained: hardcodes all shapes. kernel(**inputs) takes full inputs and
returns the full output.

Per-core pipeline (2 samples each):
  ph1: conv1 (1x1, PE f32r) + folded BN + SiLU (ACT) -> bn_stats (DVE) -> mean/var
  ph2: attention MLP as tiny matmuls; softmax normalization folded into the
       output affine (works with unnormalized exp(z) everywhere, divides by S
       at the end); exp(z) produced replicated over partitions so the
       block-diagonal weight-aggregation matmul needs no broadcasts.
  ph3: weight aggregation agg_w = sum_k e_k * W_k as a dense K=128 matmul with
       a block-diagonal lhsT (partition = (k, i_sub)), PE col-tiled.
  ph4: recompute conv1 + SiLU, then instance-norm apply into a zero-padded
       input buffer (border stays zero from a one-time memset).
  ph5: 3x3 dynamic conv as 18 accumulating matmuls (2 i-chunks x 9 taps) per
       PSUM tile, f32r at full PE rate; output affine (bn1 + bias + 1/S) on ACT.
"""

import os

import numpy as np

os.environ.setdefault("BASS_NEVER_TRACE", "1")  # no NTFF hook in this container

EPS = 1e-5
B, C1, C2, K, H, W = 16, 128, 256, 4, 80, 80
NCORES = 8
BL = B // NCORES          # samples per core
RT = 6                    # rows per spatial tile (480 cols <= 512 PSUM bank)
ROWS = [(r, min(RT, H - r)) for r in range(0, H, RT)]   # 13x6 + 1x2
NTILES = len(ROWS)
HP, WP = H + 2, W + 2     # padded 82x82
SGS = [(0, 4), (4, 4), (8, 4), (12, 2)]  # psum accumulation groups over tiles

_CACHE = {}


def _build_nc(reps=1, loop_n=None):
    from contextlib import ExitStack

    import concourse.bass as bass
    import concourse.mybir as mybir
    import concourse.tile as tile
    from concourse import bacc

    dt = mybir.dt
    AF = mybir.ActivationFunctionType
    OP = mybir.AluOpType
    f32 = dt.float32
    bf16 = dt.bfloat16

    nc = bacc.Bacc("TRN2", target_bir_lowering=False, debug=False)

    def inp(name, shape, dtype=f32):
        return nc.dram_tensor(name, shape, dtype, kind="ExternalInput").ap()

    x_d = inp("x_sh", [BL, C1, H, W], bf16)
    c1wT_d = inp("conv1wT", [C1, C2], bf16)
    # packed per-channel vectors: cols 0-1 bns, 2-3 bnb, 4-5 s1, 6-7 b1
    vec8_d = inp("vec8", [128, 8])
    # packed attn consts: 0 fc2br, 1 inv32, 2:258 dybr, 258:290 mask32, 290:298 fc1wT
    attnc_d = inp("attnc", [128, 298])
    # packed 4-partition consts: 0:128 fc2wTr, 128 fc1b
    small4_d = inp("small4", [K, 129])
    dyw_d = inp("dywsb", [128, 8, 9, C2], bf16)  # [(k,i_sub), j, tap, o]
    ones1_d = inp("ones1x128", [1, 128])
    out_d = nc.dram_tensor("out_sh", [BL, C2, H, W], f32, kind="ExternalOutput").ap()

    with tile.TileContext(nc) as tc, ExitStack() as ctx:
        pool = lambda name, bufs, **kw: ctx.enter_context(
            tc.tile_pool(name=name, bufs=bufs, **kw)
        )
        const_p = pool("consts", 1)
        ypad_p = pool("ypad", 1)
        aggw_p = pool("aggw", 2)
        stats_p = pool("stats", 4)
        mv_p = pool("mv", 4)
        vec_p = pool("vec", 8)
        attn_p = pool("attn", 2)
        xin_p = pool("xin", 2)
        stage_p = pool("stage", 4)
        outst_p = pool("outst", 3)
        cps_p = pool("cps", 4, space="PSUM")
        aps_p = pool("aps", 2, space="PSUM")
        sps_p = pool("sps", 2, space="PSUM")

        def cload(name, ap_d, shape, dtype=f32):
            t = const_p.tile(shape, dtype, tag=name)
            nc.sync.dma_start(t[:], ap_d)
            return t

        c1wT = cload("c1wT", c1wT_d, [C1, C2], bf16)
        vec8 = cload("vec8", vec8_d, [128, 8])
        attnc = cload("attnc", attnc_d, [128, 298])
        small4 = cload("small4", small4_d, [K, 129])
        ones1 = cload("ones1", ones1_d, [1, 128])
        dyw = const_p.tile([128, 8, 9, C2], bf16, tag="dyw", name="dyw")
        for j in range(8):  # split across DMA queues so x tiles aren't stuck
            nc.sync.dma_start(dyw[:, j, :, :], dyw_d[:, j, :, :])
        bns = vec8[:, 0:2]
        bnb = vec8[:, 2:4]
        s1c = vec8[:, 4:6]
        b1c = vec8[:, 6:8]
        fc2br = attnc[:, 0:1]
        inv32 = attnc[:, 1:2]
        dybr = attnc[:, 2 : 2 + C2]
        mask32 = attnc[:, 258:290]
        fc1wT = attnc[:, 290:298]
        fc2wTr = small4[:, 0:128]
        fc1b = small4[:, 128:129]

        ypads = []
        for nm in ("ypadA", "ypadB"):
            yp = ypad_p.tile([128, 2, HP, WP], bf16, tag=nm, name=nm)
            nc.gpsimd.memset(yp[:], 0.0)
            ypads.append(yp)
        epsc = const_p.tile([128, 1], f32, tag="epsc", name="epsc")
        nc.vector.memset(epsc[:], EPS)

        def phase1(b):
            """conv1 + bn + silu -> stats; silu output staged into the padded
            buffer interior (unnormalized; ph4 normalizes in place)."""
            yp = ypads[b % 2]
            stats = [stats_p.tile([128, NTILES, 6], f32, tag="stats", name="stats")
                     for _ in range(2)]
            xs = xin_p.tile([C1, H, W], bf16, tag="xs", name="xs")
            nc.sync.dma_start(xs[:], x_d[b])
            for c in range(2):
                for t, (r0, rn) in enumerate(ROWS):
                    ps = cps_p.tile([128, RT, W], f32, tag="cps", name="cps")
                    nc.tensor.matmul(
                        ps[:, :rn, :], c1wT[:, 128 * c : 128 * (c + 1)],
                        xs[:, r0 : r0 + rn, :], start=True, stop=True,
                    )
                    st = stage_p.tile([128, RT, W], bf16, tag="stage", name="stage")
                    nc.scalar.activation(
                        st[:, :rn, :], ps[:, :rn, :], AF.Silu,
                        bias=bnb[:, c : c + 1], scale=bns[:, c : c + 1],
                    )
                    nc.vector.bn_stats(
                        stats[c][:, t, :],
                        st[:, :rn, :].rearrange("p a b -> p (a b)"),
                    )
                    nc.vector.tensor_copy(
                        yp[:, c, r0 + 1 : r0 + rn + 1, 1 : W + 1], st[:, :rn, :]
                    )
            mv = []
            for c in range(2):
                m = mv_p.tile([128, 2], f32, tag="mv", name="mv")
                nc.vector.bn_aggr(m[:], stats[c][:])
                mv.append(m)
            return mv

        def phase2(b, mv):
            """attention + per-sample scalars. Returns (rstd, bd, s1S, cbias)."""
            rstd = []
            for c in range(2):
                r0 = vec_p.tile([128, 1], f32, tag="rstd", name="rstd")
                nc.scalar.activation(r0[:], mv[c][:, 1:2], AF.Sqrt, bias=epsc[:])
                nc.vector.reciprocal(r0[:], r0[:])
                rstd.append(r0)
            a_ps = sps_p.tile([K, 1], f32, tag="sps", name="attn_ps")
            for c in range(2):
                nc.tensor.matmul(
                    a_ps[:], fc1wT[:, K * c : K * (c + 1)], mv[c][:, 0:1],
                    start=(c == 0), stop=(c == 1),
                )
            a_sb = attn_p.tile([K, 1], f32, tag="a_sb", name="a_sb")
            nc.scalar.activation(a_sb[:], a_ps[:], AF.Relu, bias=fc1b[:])
            z_ps = sps_p.tile([128, 1], f32, tag="sps", name="z_ps")
            nc.tensor.matmul(z_ps[:], fc2wTr[:], a_sb[:], start=True, stop=True)
            e_bc = attn_p.tile([128, 1], f32, tag="e_bc", name="e_bc")  # exp(z)[p//32]
            nc.scalar.activation(e_bc[:], z_ps[:], AF.Exp, bias=fc2br[:])
            # block-diagonal lhsT for weight aggregation
            bd = attn_p.tile([128, 32], bf16, tag="bd", name="bd")
            nc.vector.tensor_scalar_mul(bd[:], mask32[:], e_bc[:])
            # S = sum_k e_k  (partition reduction via matmul; inv32 cancels x32)
            S_ps = sps_p.tile([1, 1], f32, tag="sps", name="S_ps")
            nc.tensor.matmul(S_ps[:], e_bc[:], inv32[:], start=True, stop=True)
            rS = attn_p.tile([1, 1], f32, tag="rS", name="rS")
            nc.vector.reciprocal(rS[:], S_ps[:])
            # broadcast 1/S to all partitions via K=1 matmul
            rb_ps = sps_p.tile([128, 1], f32, tag="sps", name="rb_ps")
            nc.tensor.matmul(rb_ps[:], ones1[:], rS[:], start=True, stop=True)
            # agg bias: aggb[:, c] = sum_k e_k dy_b[k, 128c + :]
            aggb_ps = sps_p.tile([128, 2], f32, tag="sps", name="aggb_ps")
            for c in range(2):
                nc.tensor.matmul(
                    aggb_ps[:, c : c + 1], dybr[:, 128 * c : 128 * (c + 1)],
                    e_bc[:], start=True, stop=True,
                )
            s1S = vec_p.tile([128, 2], f32, tag="s1S", name="s1S")
            cbias = vec_p.tile([128, 2], f32, tag="cbias", name="cbias")
            for c in range(2):
                nc.vector.tensor_scalar_mul(
                    s1S[:, c : c + 1], s1c[:, c : c + 1], rb_ps[:]
                )
                nc.vector.scalar_tensor_tensor(
                    cbias[:, c : c + 1], aggb_ps[:, c : c + 1],
                    s1S[:, c : c + 1], b1c[:, c : c + 1],
                    op0=OP.mult, op1=OP.add,
                )
            return rstd, bd, s1S, cbias

        def phase3(b, bd):
            """agg_w = blockdiag(e) @ dyw -> aggW [128, ichunk, tap, o]"""
            aggW = aggw_p.tile([128, 2, 9, C2], bf16, tag="aggW", name="aggW")
            for c in range(2):
                for t0, tn in ((0, 2), (2, 2), (4, 2), (6, 2), (8, 1)):
                    agps = aps_p.tile([128, tn, C2], f32, tag="agps", name="agps")
                    for jj in range(4):
                        j = 4 * c + jj
                        nc.tensor.matmul(
                            agps[32 * jj : 32 * jj + 32, :, :],
                            bd[:],
                            dyw[:, j, t0 : t0 + tn, :],
                            start=True, stop=True,
                            tile_position=(0, 32 * jj),
                        )
                    nc.scalar.activation(
                        aggW[:, c, t0 : t0 + tn, :], agps[:], AF.Copy
                    )
            return aggW

        def phase4(b, mv, rstd):
            """instance-norm apply in place: int = (int - mu) * rstd (DVE only)."""
            for c in range(2):
                inter = ypads[b % 2][:, c, 1 : H + 1, 1 : W + 1]
                nc.vector.tensor_scalar(
                    inter, inter, mv[c][:, 0:1], rstd[c][:],
                    op0=OP.subtract, op1=OP.mult,
                )

        def phase5(b, aggW, s1S, cbias):
            """dynamic 3x3 conv + output affine + store."""
            for oc in range(2):
                for g0, gn in SGS:
                    pss = [cps_p.tile([128, RT, W], f32, tag="cps", name="cps")
                           for _ in range(gn)]
                    first, last = (0, 0, 0), (1, 2, 2)
                    for c in range(2):
                        for ty in range(3):
                            for tx in range(3):
                                lhsT = aggW[:, c, 3 * ty + tx,
                                            128 * oc : 128 * (oc + 1)]
                                for ti in range(gn):
                                    r0, rn = ROWS[g0 + ti]
                                    rhs = ypads[b % 2][:, c, r0 + ty : r0 + ty + rn,
                                                       tx : tx + W]
                                    nc.tensor.matmul(
                                        pss[ti][:, :rn, :], lhsT, rhs,
                                        start=((c, ty, tx) == first),
                                        stop=((c, ty, tx) == last),
                                    )
                    for ti in range(gn):
                        r0, rn = ROWS[g0 + ti]
                        ot = outst_p.tile([128, RT, W], f32, tag="outst", name="outst")
                        nc.scalar.activation(
                            ot[:, :rn, :], pss[ti][:, :rn, :], AF.Identity,
                            bias=cbias[:, oc : oc + 1], scale=s1S[:, oc : oc + 1],
                        )
                        nc.sync.dma_start(
                            out_d[b, 128 * oc : 128 * (oc + 1), r0 : r0 + rn, :],
                            ot[:, :rn, :],
                        )

        # --- schedule ---
        def body():
            for _rep in range(reps):
                mvs = {b: phase1(b) for b in range(BL)}
                state = {}
                for b in range(BL):
                    rstd, bd, s1S, cbias = phase2(b, mvs[b])
                    phase4(b, mvs[b], rstd)
                    aggW = phase3(b, bd)
                    state[b] = (aggW, s1S, cbias)
                for b in range(BL):
                    aggW, s1S, cbias = state[b]
                    phase5(b, aggW, s1S, cbias)

        if loop_n is None:
            body()
        else:
            with tc.For_i(0, loop_n, 1):
                body()

    if os.environ.get('LDW_DEDUPE', '0') == '1':
        _dedupe_ldweights(nc, mybir)
    if not nc.is_finalized():
        nc.finalize()  # Bacc.compile(): wait legalization, act-table loads, DCE
    return nc


def _ldw_key(inst):
    ap = inst.ins[0]
    mloc = getattr(ap, "memorylocation", None)
    mname = getattr(mloc, "name", None) if mloc is not None else None
    return (mname, getattr(ap, "offset", None), str(getattr(ap, "ap", None)),
            getattr(ap, "dtype", None), inst.perf_mode, inst.is_transpose,
            inst.tile_position)


def _dedupe_ldweights(nc, mybir):
    """Drop an InstLdweights when the PE array already holds the same weights
    (same AP, only plain matmuls in between, no sync side effects)."""
    removed = 0
    for f in nc.m.functions:
        for bb in f.blocks:
            last_key = None
            keep = []
            for inst in bb.instructions:
                if isinstance(inst, mybir.InstLdweights):
                    si = inst.sync_info
                    clean = not si or (not si.on_wait and not si.on_update)
                    key = _ldw_key(inst)
                    if clean and key == last_key:
                        removed += 1
                        continue
                    last_key = key
                elif isinstance(inst, mybir.InstMatmult):
                    if inst.is_transpose:
                        last_key = None
                elif getattr(inst, "engine", None) == mybir.EngineType.PE:
                    last_key = None
                keep.append(inst)
            bb.instructions[:] = keep
    return removed


def _host_prep(inputs):
    f = np.float32
    conv1_w = np.asarray(inputs["conv1_w"], f)
    bns = (np.asarray(inputs["bn_g"], f) / np.sqrt(np.asarray(inputs["bn_v"], f) + EPS))
    bnb = np.asarray(inputs["bn_b"], f) - np.asarray(inputs["bn_m"], f) * bns
    s1 = (np.asarray(inputs["bn1_g"], f) / np.sqrt(np.asarray(inputs["bn1_v"], f) + EPS))
    b1 = np.asarray(inputs["bn1_b"], f) - np.asarray(inputs["bn1_m"], f) * s1
    fc1_w = np.asarray(inputs["fc1_w"], f)
    fc2_w = np.asarray(inputs["fc2_w"], f)
    dy_w = np.asarray(inputs["dy_w"], f)
    dy_b = np.asarray(inputs["dy_b"], f)

    import ml_dtypes

    bf = ml_dtypes.bfloat16
    t = dy_w.transpose(0, 2, 3, 4, 1)              # [k, i, ty, tx, o]
    t = t.reshape(K, 8, 32, 3, 3, C2)              # [k, j, i_sub, ty, tx, o]
    dyw_sb = np.ascontiguousarray(
        t.transpose(0, 2, 1, 3, 4, 5).reshape(128, 8, 9, C2).astype(bf)
    )
    mask = np.zeros((128, 32), f)
    mask[np.arange(128), np.arange(128) % 32] = 1.0
    ks = np.arange(128) // 32
    vec8 = np.concatenate(
        [bns.reshape(2, 128).T, bnb.reshape(2, 128).T,
         s1.reshape(2, 128).T, b1.reshape(2, 128).T], axis=1,
    ).astype(f)
    fc1wT = (fc1_w.T.reshape(2, 128, K).transpose(1, 0, 2).reshape(128, 2 * K))
    attnc = np.concatenate(
        [np.asarray(inputs["fc2_b"], f)[ks].reshape(128, 1),   # 0 fc2br
         np.full((128, 1), 1.0 / 32.0, f),                     # 1 inv32
         (dy_b[ks, :] / 32.0).astype(f),                       # 2:258 dybr
         mask,                                                 # 258:290
         fc1wT.astype(f)], axis=1,                             # 290:298
    )
    small4 = np.concatenate(
        [fc2_w[ks, :].T[:, :].astype(f)[:, :128],              # 0:128 fc2wTr
         np.asarray(inputs["fc1_b"], f).reshape(K, 1)], axis=1,
    )
    consts = {
        "conv1wT": np.ascontiguousarray(conv1_w.T.astype(bf)),      # [C1, C2]
        "vec8": np.ascontiguousarray(vec8),
        "attnc": np.ascontiguousarray(attnc),
        "small4": np.ascontiguousarray(small4),
        "dywsb": dyw_sb,
        "ones1x128": np.ones((1, 128), f),
    }
    return consts


def _make_in_maps(inputs):
    import ml_dtypes

    x = np.ascontiguousarray(
        np.asarray(inputs["x"], np.float32).astype(ml_dtypes.bfloat16)
    )
    consts = _host_prep(inputs)
    in_maps = []
    for core in range(NCORES):
        m = {"x_sh": np.ascontiguousarray(x[core * BL : (core + 1) * BL])}
        m.update(consts)
        in_maps.append(m)
    return in_maps


def kernel(**inputs):
    from concourse.bass_utils import run_bass_kernel_spmd

    if "nc" not in _CACHE:
        _CACHE["nc"] = _build_nc()
    nc = _CACHE["nc"]

    in_maps = _make_in_maps(inputs)

    res = run_bass_kernel_spmd(nc, in_maps, core_ids=list(range(NCORES)))
    globals()["_LAST_RESULTS"] = res
    out = np.concatenate([r["out_sh"] for r in res.results], axis=0)
    return out
